# revision 26
# baseline (speedup 1.0000x reference)
"""Trainium2 Bass kernel for nn_ExpertsLinearEnsemble.

Reference computation (B=16384, D=768, E=124, C=6):
  expert_logits  = Mlp_cls(x).reshape(B, E, C)     # D -> D -> gelu -> E*C
  ew_logits      = Mlp_ew(x)                       # D -> D -> gelu -> E
  which_expert   = Mlp_we(x)                       # D -> D -> gelu -> E
  n = clamp(n_experts, E); thr = n-th largest of which_expert per row
  mask out experts with which_expert < thr; softmax ew_logits over kept
  experts; softmax expert_logits over classes; combined = sum_e w_e *
  proba_e / sum_e w_e.

Strategy (pure data parallel, 2048 rows/core):
  - Host transposes x so the contraction dim D sits on SBUF partitions;
    the whole device pipeline runs feature-major ([feature, row] tiles),
    so matmul outputs feed the next layer with no on-chip transposes of
    activations.
  - Precision split: the which_expert path must reproduce the fp32
    ordering of the reference (the top-n mask is a hard threshold), so
    it runs in plain fp32 matmuls.  The cls/ew paths and the small
    combine matmuls only need ~1e-3 relative accuracy and run in bf16
    (fp32 PSUM accumulation).
  - Top-n threshold per row: rows are sorted by n on host and dealt
    round-robin to cores, so each 128-row subtile has a narrow n-range
    and a fixed number of max8/match_replace rounds (descending sort for
    small n, ascending for large n) suffices.  The threshold is the
    (n-1)-th sorted value, extracted with a one-hot dot against a host
    supplied selector index; mask = which_expert >= thr matches the
    reference's `which_expert < thr` masking exactly, ties included.
  - Class softmax sums / per-expert broadcasts / final combine run as
    tiny PE matmuls against constant group matrices; the expert-weight
    sum (softmax denominator, which algebraically cancels against the
    final division) accumulates in a separate row and one divide at the
    end normalizes everything.
"""

import os
import sys

for _p in ("/opt/trn_rl_repo", "/root/.axon_site/_ro/trn_rl_repo"):
    if os.path.isdir(_p) and _p not in sys.path:
        sys.path.insert(0, _p)

import numpy as np

B, D, E, C = 16384, 768, 124, 6
EC = E * C            # 744
NCORES = 8
BC = B // NCORES      # 2048 rows per core
RT = 512              # rows per macro tile (PSUM bank = 512 fp32)
NT = BC // RT         # 4 macro tiles per core
SUB = 128             # rows per sort subtile
NS = BC // SUB        # 16 subtiles per core
KT = D // 128         # 6 contraction tiles
MT1 = D // 128        # 6 output tiles for layer 1
MT2 = EC // E         # 6 output tiles of 124 for the cls head

# Fixed per-subtile sort schedule.  Subtile s holds rows whose global
# sorted-n positions are [1024 s, 1024 (s+1)); with n ~ U[1,124] the
# boundary quantiles sit many sigma inside these capabilities.
# s in [0, 8): descending sort, handles n <= 8 R; s in [8, 16): ascending
# (sort of -we), handles n >= 125 - 8 R.
R_DESC = [2, 3, 4, 5, 6, 7, 8, 9]
R_ASC = [9, 8, 7, 6, 5, 4, 3, 2]
SUB_DIR = [True] * 8 + [False] * 8         # True = descending
SUB_R = R_DESC + R_ASC
FALLBACK_R = [16] * NS                     # safe for any n distribution
FALLBACK_DIR = [True] * NS

NEG_FILL = -1.0e30

_BUILD_CACHE = {}


def _build_nc(sub_dir, sub_r, act="Gelu"):
    """Build the (SPMD, per-core) Bass program.  Data independent."""
    from contextlib import ExitStack

    import concourse.mybir as mybir
    import concourse.tile as tile
    from concourse import bacc

    dt = mybir.dt
    AF = mybir.ActivationFunctionType
    OP = mybir.AluOpType
    f32 = dt.float32
    f32r = dt.float32r
    bf16 = dt.bfloat16

    nc = bacc.Bacc(
        "TRN2",
        target_bir_lowering=False,
        debug=False,
        enable_asserts=False,
        num_devices=NCORES,
    )

    def din(name, shape, dtype=f32):
        return nc.dram_tensor(name, list(shape), dtype, kind="ExternalInput")

    xtb_d = din("xtb", [D, BC], bf16)           # bf16 x.T (hi part, all paths)
    xlo_d = din("xlo", [D, BC], bf16)           # bf16 residual x.T - hi (we path)
    ksel_d = din("ksel", [SUB, NS])
    w1c_d = din("w1c", [D, D], bf16)
    w1e_d = din("w1e", [D, D], bf16)
    w1wh_d = din("w1wh", [D, D], bf16)          # we w1 hi
    w1wl_d = din("w1wl", [D, D], bf16)          # we w1 lo residual
    b1_d = {m: din(f"b1{m}", [128, MT1]) for m in "cwe"}
    w2c_d = din("w2c", [D, EC], bf16)           # class-major column order
    w2e_d = din("w2e", [D, E], bf16)
    w2w_d = din("w2w", [D, E])                  # fp32
    b2c_d = din("b2c", [E, MT2])                # [e, class]
    b2w_d = din("b2w", [E, 1])
    b2e_d = din("b2e", [E, 1])
    hmat_d = din("hmat", [E, MT2, C], bf16)
    ident_d = din("ident", [128, 128])
    iota_d = din("iota", [128, 128])
    out_d = nc.dram_tensor("out", [C, BC], f32, kind="ExternalOutput")

    with tile.TileContext(nc) as tc, ExitStack() as ctx:
        const = ctx.enter_context(tc.tile_pool(name="const", bufs=1))
        xtp = ctx.enter_context(tc.tile_pool(name="xtp", bufs=2))
        hp = ctx.enter_context(tc.tile_pool(name="hp", bufs=2))
        epp = ctx.enter_context(tc.tile_pool(name="epp", bufs=2))
        wep = ctx.enter_context(tc.tile_pool(name="wep", bufs=2))
        sp = ctx.enter_context(tc.tile_pool(name="sp", bufs=2))
        wp = ctx.enter_context(tc.tile_pool(name="wp", bufs=2))
        psmm = ctx.enter_context(tc.tile_pool(name="psmm", bufs=4, space="PSUM"))
        pstr = ctx.enter_context(tc.tile_pool(name="pstr", bufs=1, space="PSUM"))
        psmask = ctx.enter_context(tc.tile_pool(name="psmask", bufs=1, space="PSUM"))
        pss = ctx.enter_context(tc.tile_pool(name="pss", bufs=1, space="PSUM"))
        psout = ctx.enter_context(tc.tile_pool(name="psout", bufs=1, space="PSUM"))

        # ---- resident constants / weights -------------------------------
        # Weights ride the gpsimd (SWDGE) queues so the per-tile x DMAs on
        # the sync (HWDGE) queues are not stuck behind ~10 MB of weights;
        # the first layer-1 matmul only needs w1c + the first x tile.
        # Split per k-tile so the first matmul only waits on its own slice.
        def load_w(dram, cols, dtype, tag):
            t = const.tile([128, KT, cols], dtype, tag=tag)
            ap = dram.ap().rearrange("(ko p) m -> p ko m", p=128)
            for k in range(KT):
                nc.gpsimd.dma_start(t[:, k, :], ap[:, k, :])
            return t

        def load_c(dram, shape, dtype, tag):
            t = const.tile(shape, dtype, tag=tag)
            nc.gpsimd.dma_start(t[:], dram.ap())
            return t

        w1sb = {"c": load_w(w1c_d, D, bf16, "w1c")}
        b1sb = {m: load_c(b1_d[m], [128, MT1], f32, f"b1{m}") for m in "cwe"}
        w1sb["wh"] = load_w(w1wh_d, D, bf16, "w1wh")
        w1sb["wl"] = load_w(w1wl_d, D, bf16, "w1wl")
        w1sb["e"] = load_w(w1e_d, D, bf16, "w1e")
        w2sb = {
            "c": load_w(w2c_d, EC, bf16, "w2c"),
            "w": load_w(w2w_d, E, f32, "w2w"),
            "e": load_w(w2e_d, E, bf16, "w2e"),
        }
        b2csb = load_c(b2c_d, [E, MT2], f32, "b2c")
        b2wsb = load_c(b2w_d, [E, 1], f32, "b2w")
        b2esb = load_c(b2e_d, [E, 1], f32, "b2e")
        hmat = load_c(hmat_d, [E, MT2, C], bf16, "hmat")
        ident = load_c(ident_d, [128, 128], f32, "ident")
        iotam = load_c(iota_d, [128, 128], f32, "iota")
        kselsb = load_c(ksel_d, [SUB, NS], f32, "ksel")
        ones6 = const.tile([1, C], f32, tag="ones6")
        nc.vector.memset(ones6[:], 1.0)
        ones124 = const.tile([E, 1], bf16, tag="ones124")
        nc.vector.memset(ones124[:], 1.0)
        outacc = const.tile([C, BC], f32, tag="outacc")

        xtb_ap = xtb_d.ap().rearrange("(ko p) n -> p ko n", p=128)
        xlo_ap = xlo_d.ap().rearrange("(ko p) n -> p ko n", p=128)

        for T in range(NT):
            rs = slice(T * RT, (T + 1) * RT)
            xtb = xtp.tile([128, KT, RT], bf16, tag="xtb")
            nc.sync.dma_start(xtb[:], xtb_ap[:, :, rs])
            xlo = xtp.tile([128, KT, RT], bf16, tag="xlo")
            nc.sync.dma_start(xlo[:], xlo_ap[:, :, rs])

            # ---- three MLPs: all layer-1s (one gelu table set), then all
            # layer-2s (one exp table set) ---------------------------------
            expP = epp.tile([E, MT2, RT], bf16, tag="expP")
            weT = wep.tile([E, RT], f32, tag="weT")
            expew = wep.tile([E, RT], f32, tag="expew")
            hts = {}
            for m in "cwe":
                fp = m == "w"
                ht = hp.tile([128, KT, RT], f32 if fp else bf16, tag=f"ht{m}")
                for mt in range(MT1):
                    cs = slice(mt * 128, (mt + 1) * 128)
                    ps = psmm.tile([128, RT], f32, tag="psmm")
                    if fp:
                        # hi/lo split: full-fp32-accuracy product via three
                        # bf16 matmuls (whi@xhi + whi@xlo + wlo@xhi)
                        for k in range(KT):
                            nc.tensor.matmul(
                                ps[:], w1sb["wh"][:, k, cs], xtb[:, k, :],
                                start=(k == 0), stop=False,
                            )
                            nc.tensor.matmul(
                                ps[:], w1sb["wh"][:, k, cs], xlo[:, k, :],
                                start=False, stop=False,
                            )
                            nc.tensor.matmul(
                                ps[:], w1sb["wl"][:, k, cs], xtb[:, k, :],
                                start=False, stop=(k == KT - 1),
                            )
                    else:
                        for k in range(KT):
                            nc.tensor.matmul(
                                ps[:], w1sb[m][:, k, cs], xtb[:, k, :],
                                start=(k == 0), stop=(k == KT - 1),
                            )
                    nc.scalar.activation(
                        ht[:, mt, :], ps[:], getattr(AF, act),
                        bias=b1sb[m][:, mt : mt + 1],
                    )
                hts[m] = ht
            for t2 in range(MT2):
                ps = psmm.tile([128, RT], f32, tag="psmm")
                for k in range(KT):
                    nc.tensor.matmul(
                        ps[:E],
                        w2sb["c"][:, k, t2 * E : (t2 + 1) * E],
                        hts["c"][:, k, :],
                        start=(k == 0),
                        stop=(k == KT - 1),
                    )
                nc.scalar.activation(
                    expP[:, t2, :], ps[:E], AF.Exp, bias=b2csb[:, t2 : t2 + 1]
                )
            for m in "we":
                ps = psmm.tile([128, RT], f32, tag="psmm")
                for k in range(KT):
                    nc.tensor.matmul(
                        ps[:E],
                        w2sb[m][:, k, :],
                        hts[m][:, k, :],
                        start=(k == 0),
                        stop=(k == KT - 1),
                    )
                if m == "w":
                    nc.scalar.activation(weT[:], ps[:E], AF.Identity, bias=b2wsb[:])
                else:
                    nc.scalar.activation(expew[:], ps[:E], AF.Exp, bias=b2esb[:])

            # ---- per-row top-n mask (row-major subtiles) ----------------
            maskT = psmask.tile([E, RT], f32, tag="maskT")
            for j in range(RT // SUB):
                s = (RT // SUB) * T + j
                Rr, desc = sub_r[s], sub_dir[s]
                cs = slice(j * SUB, (j + 1) * SUB)
                trp = pstr.tile([128, 128], f32, tag="trp")
                nc.tensor.transpose(trp[:, :E], weT[:, cs], ident[:E, :E])
                weRow = sp.tile([128, E], f32, tag="weRow")
                nc.scalar.copy(weRow[:], trp[:, :E])
                scratch = sp.tile([128, E], f32, tag="scratch")
                if desc:
                    nc.vector.tensor_copy(scratch[:], weRow[:])
                else:
                    nc.vector.tensor_scalar_mul(scratch[:], weRow[:], -1.0)
                srt = sp.tile([128, 128], f32, tag="srt")
                for r in range(Rr):
                    nc.vector.max(out=srt[:, 8 * r : 8 * r + 8], in_=scratch[:])
                    if r < Rr - 1:
                        nc.vector.match_replace(
                            out=scratch[:],
                            in_to_replace=srt[:, 8 * r : 8 * r + 8],
                            in_values=scratch[:],
                            imm_value=NEG_FILL,
                        )
                w8 = 8 * Rr
                ohtmp = sp.tile([128, 128], f32, tag="ohtmp")
                thr = sp.tile([128, 1], f32, tag="thr")
                nc.vector.scalar_tensor_tensor(
                    out=ohtmp[:, :w8],
                    in0=iotam[:, :w8],
                    scalar=kselsb[:, s : s + 1],
                    in1=srt[:, :w8],
                    op0=OP.is_equal,
                    op1=OP.mult,
                    accum_out=thr[:],
                )
                if not desc:
                    nc.vector.tensor_scalar_mul(thr[:], thr[:], -1.0)
                maskRow = sp.tile([128, E], f32, tag="maskRow")
                nc.vector.tensor_scalar(maskRow[:], weRow[:], thr[:], None, OP.is_ge)
                nc.tensor.transpose(maskT[:, cs], maskRow[:], ident[:])

            # ---- combine (class-major expP: tile t2 = class c) ----------
            wT = wp.tile([E, RT], bf16, tag="wT")
            nc.vector.tensor_tensor(wT[:], expew[:], maskT[:], OP.mult)
            # S[e, r] = sum_c exp(logit_{e,c,r}) via a DVE add tree
            s01 = wp.tile([E, RT], f32, tag="s01")
            nc.vector.tensor_tensor(s01[:], expP[:, 0, :], expP[:, 1, :], OP.add)
            s23 = wp.tile([E, RT], f32, tag="s23")
            nc.vector.tensor_tensor(s23[:], expP[:, 2, :], expP[:, 3, :], OP.add)
            s45 = wp.tile([E, RT], f32, tag="s45")
            nc.vector.tensor_tensor(s45[:], expP[:, 4, :], expP[:, 5, :], OP.add)
            s0123 = wp.tile([E, RT], f32, tag="s0123")
            nc.vector.tensor_tensor(s0123[:], s01[:], s23[:], OP.add)
            S_sb = wp.tile([E, RT], f32, tag="S_sb")
            nc.vector.tensor_tensor(S_sb[:], s0123[:], s45[:], OP.add)
            den_ps = pss.tile([E, RT], f32, tag="S")
            nc.tensor.matmul(den_ps[:1, :], ones124[:], wT[:], start=True, stop=True)
            Sr = wp.tile([E, RT], f32, tag="Sr")
            nc.vector.reciprocal_approx_fast(Sr[:], S_sb[:])
            u = wp.tile([E, RT], bf16, tag="u")
            nc.vector.tensor_tensor(u[:], wT[:], Sr[:], OP.mult)
            out_ps = psout.tile([C, RT], f32, tag="out")
            for t2 in range(MT2):
                wexp = wp.tile([E, RT], bf16, tag="wexp")
                nc.vector.tensor_tensor(wexp[:], expP[:, t2, :], u[:], OP.mult)
                nc.tensor.matmul(
                    out_ps[:],
                    hmat[:, t2, :],
                    wexp[:],
                    start=(t2 == 0),
                    stop=(t2 == MT2 - 1),
                )
            nc.scalar.copy(outacc[:, rs], out_ps[:])

            # normalize by the expert-weight sum and ship this tile's rows
            den_sb = wp.tile([1, RT], f32, tag="den_sb")
            nc.scalar.copy(den_sb[:], den_ps[:1, :])
            recipd = wp.tile([1, RT], f32, tag="recipd")
            nc.vector.reciprocal_approx_fast(recipd[:], den_sb[:])
            rep = pss.tile([E, RT], f32, tag="S")
            nc.tensor.matmul(rep[:C, :], ones6[:], recipd[:], start=True, stop=True)
            nc.vector.tensor_tensor(outacc[:, rs], outacc[:, rs], rep[:C, :], OP.mult)
            nc.sync.dma_start(out_d.ap()[:, rs], outacc[:, rs])

    nc.compile()
    return nc


def _get_nc(sub_dir, sub_r, act="Gelu"):
    key = (tuple(sub_dir), tuple(sub_r), act)
    if key not in _BUILD_CACHE:
        _BUILD_CACHE[key] = _build_nc(sub_dir, sub_r, act)
    return _BUILD_CACHE[key]


def _host_prep(x, n_experts):
    n = np.minimum(np.asarray(n_experts).astype(np.int64), E).astype(np.int32)
    order = np.argsort(n, kind="stable")
    ns_sorted = n[order]

    sub_dir, sub_r = SUB_DIR, SUB_R
    ok = True
    for s in range(NS):
        lo = int(ns_sorted[(B // NS) * s])
        hi = int(ns_sorted[(B // NS) * (s + 1) - 1])
        if sub_dir[s]:
            ok &= hi <= 8 * sub_r[s]
        else:
            ok &= lo >= E + 1 - 8 * sub_r[s]
    if not ok:
        sub_dir, sub_r = FALLBACK_DIR, FALLBACK_R

    rows_by_core = [order[c::NCORES] for c in range(NCORES)]
    import ml_dtypes

    bf16 = ml_dtypes.bfloat16
    xts, ksels = [], []
    for c in range(NCORES):
        rows = rows_by_core[c]
        xt = np.ascontiguousarray(x[rows].T.astype(np.float32))
        xhi = xt.astype(bf16)
        xlo = (xt - xhi.astype(np.float32)).astype(bf16)
        xts.append((xhi, xlo))
        nv = n[rows].astype(np.float32)
        ks = np.empty(BC, np.float32)
        for s in range(NS):
            seg = slice(SUB * s, SUB * (s + 1))
            ks[seg] = (nv[seg] - 1.0) if sub_dir[s] else (E - nv[seg])
        ksels.append(np.ascontiguousarray(ks.reshape(NS, SUB).T))
    return rows_by_core, xts, ksels, sub_dir, sub_r


def _host_consts():
    # class-major cls tiling: output tile t holds columns f' = t*E + e,
    # which map to original cls column e*C + t (expert e, class t)
    hmat = np.zeros((E, MT2, C), np.float32)
    for t in range(MT2):
        hmat[:, t, t] = 1.0
    ident = np.eye(128, dtype=np.float32)
    iota = np.broadcast_to(np.arange(128, dtype=np.float32), (128, 128)).copy()
    return hmat, ident, iota


def _host_inputs(inputs):
    """All DRAM input arrays except the per-core xt/ksel."""
    import ml_dtypes

    bf16 = ml_dtypes.bfloat16
    hmat, ident, iota = _host_consts()
    f32 = np.float32
    w1w = np.asarray(inputs["we_w1"], f32)
    w1wh = w1w.astype(bf16)
    w1wl = (w1w - w1wh.astype(f32)).astype(bf16)
    # permute cls layer-2 columns to class-major: new col t*E+e <- e*C+t
    cidx = (np.arange(C)[:, None] + (np.arange(E) * C)[None, :]).ravel()
    w2c = np.asarray(inputs["cls_w2"], f32)[:, cidx]
    b2c = np.asarray(inputs["cls_b2"], f32)[cidx]
    return {
        "w1c": np.asarray(inputs["cls_w1"], f32).astype(bf16),
        "w1wh": w1wh,
        "w1wl": w1wl,
        "w1e": np.asarray(inputs["ew_w1"], f32).astype(bf16),
        "b1c": np.ascontiguousarray(np.asarray(inputs["cls_b1"], f32).reshape(MT1, 128).T),
        "b1w": np.ascontiguousarray(np.asarray(inputs["we_b1"], f32).reshape(MT1, 128).T),
        "b1e": np.ascontiguousarray(np.asarray(inputs["ew_b1"], f32).reshape(MT1, 128).T),
        "w2c": w2c.astype(bf16),
        "w2w": np.asarray(inputs["we_w2"], f32),
        "w2e": np.asarray(inputs["ew_w2"], f32).astype(bf16),
        "b2c": np.ascontiguousarray(b2c.reshape(MT2, E).T),
        "b2w": np.asarray(inputs["we_b2"], f32).reshape(E, 1),
        "b2e": np.asarray(inputs["ew_b2"], f32).reshape(E, 1),
        "hmat": hmat.astype(bf16),
        "ident": ident,
        "iota": iota,
    }


def _per_core_inputs(xts, ksels, c):
    return {
        "xtb": xts[c][0],
        "xlo": xts[c][1],
        "ksel": ksels[c],
    }


def kernel(**inputs):
    x = np.asarray(inputs["x"], np.float32)
    rows_by_core, xts, ksels, sub_dir, sub_r = _host_prep(x, inputs["n_experts"])
    shared = _host_inputs(inputs)
    in_maps = [
        {**shared, **_per_core_inputs(xts, ksels, c)} for c in range(NCORES)
    ]

    nc = _get_nc(sub_dir, sub_r)

    from concourse.bass_utils import run_bass_kernel_spmd

    res = run_bass_kernel_spmd(nc, in_maps, core_ids=list(range(NCORES)))

    full = np.empty((B, C), np.float32)
    for c in range(NCORES):
        full[rows_by_core[c]] = res.results[c]["out"].T
    return full


if __name__ == "__main__":
    print("smoke build only")
    _get_nc(SUB_DIR, SUB_R)
    print("built ok")



# revision 34
# speedup vs baseline: 1.2918x; 1.2918x over previous
"""Trainium2 Bass kernel for nn_ExpertsLinearEnsemble.

Reference computation (B=16384, D=768, E=124, C=6):
  expert_logits  = Mlp_cls(x).reshape(B, E, C)     # D -> D -> gelu -> E*C
  ew_logits      = Mlp_ew(x)                       # D -> D -> gelu -> E
  which_expert   = Mlp_we(x)                       # D -> D -> gelu -> E
  n = clamp(n_experts, E); thr = n-th largest of which_expert per row
  mask out experts with which_expert < thr; softmax ew_logits over kept
  experts; softmax expert_logits over classes; combined = sum_e w_e *
  proba_e / sum_e w_e.

Strategy (pure data parallel, 2048 rows/core):
  - Host transposes x so the contraction dim D sits on SBUF partitions;
    the whole device pipeline runs feature-major ([feature, row] tiles),
    so matmul outputs feed the next layer with no on-chip transposes of
    activations.
  - Precision split: the which_expert path must reproduce the fp32
    ordering of the reference (the top-n mask is a hard threshold), so
    it runs in plain fp32 matmuls.  The cls/ew paths and the small
    combine matmuls only need ~1e-3 relative accuracy and run in bf16
    (fp32 PSUM accumulation).
  - Top-n threshold per row: rows are sorted by n on host and dealt
    round-robin to cores, so each 128-row subtile has a narrow n-range
    and a fixed number of max8/match_replace rounds (descending sort for
    small n, ascending for large n) suffices.  The threshold is the
    (n-1)-th sorted value, extracted with a one-hot dot against a host
    supplied selector index; mask = which_expert >= thr matches the
    reference's `which_expert < thr` masking exactly, ties included.
  - Class softmax sums / per-expert broadcasts / final combine run as
    tiny PE matmuls against constant group matrices; the expert-weight
    sum (softmax denominator, which algebraically cancels against the
    final division) accumulates in a separate row and one divide at the
    end normalizes everything.
"""

import os
import sys

for _p in ("/opt/trn_rl_repo", "/root/.axon_site/_ro/trn_rl_repo"):
    if os.path.isdir(_p) and _p not in sys.path:
        sys.path.insert(0, _p)

import numpy as np

B, D, E, C = 16384, 768, 124, 6
EC = E * C            # 744
NCORES = 8
BC = B // NCORES      # 2048 rows per core
RT = 512              # rows per macro tile (PSUM bank = 512 fp32)
NT = BC // RT         # 4 macro tiles per core
SUB = 128             # rows per sort subtile
NS = BC // SUB        # 16 subtiles per core
KT = D // 128         # 6 contraction tiles
MT1 = D // 128        # 6 output tiles for layer 1
MT2 = EC // E         # 6 output tiles of 124 for the cls head

# Fixed per-subtile sort schedule.  Subtile s holds rows whose global
# sorted-n positions are [1024 s, 1024 (s+1)); with n ~ U[1,124] the
# boundary quantiles sit many sigma inside these capabilities.
# s in [0, 8): descending sort, handles n <= 8 R; s in [8, 16): ascending
# (sort of -we), handles n >= 125 - 8 R.
R_DESC = [2, 3, 4, 5, 6, 7, 8, 9]
R_ASC = [9, 8, 7, 6, 5, 4, 3, 2]
SUB_DIR = [True] * 8 + [False] * 8         # True = descending
SUB_R = R_DESC + R_ASC
FALLBACK_R = [16] * NS                     # safe for any n distribution
FALLBACK_DIR = [True] * NS

NEG_FILL = -1.0e30

_BUILD_CACHE = {}


def _build_nc(sub_dir, sub_r, act="Gelu"):
    """Build the (SPMD, per-core) Bass program.  Data independent."""
    from contextlib import ExitStack

    import concourse.mybir as mybir
    import concourse.tile as tile
    from concourse import bacc

    dt = mybir.dt
    AF = mybir.ActivationFunctionType
    OP = mybir.AluOpType
    f32 = dt.float32
    f32r = dt.float32r
    bf16 = dt.bfloat16

    nc = bacc.Bacc(
        "TRN2",
        target_bir_lowering=False,
        debug=False,
        enable_asserts=False,
        num_devices=NCORES,
    )

    def din(name, shape, dtype=f32):
        return nc.dram_tensor(name, list(shape), dtype, kind="ExternalInput")

    fp8 = dt.float8e4
    EW_WS = 16.0                                # ew weight pre-scale (fp8 range)

    xtb_d = din("xtb", [D, BC], bf16)           # bf16 x.T (hi part, all paths)
    xlo_d = din("xlo", [D, BC], bf16)           # bf16 residual x.T - hi (we path)
    x8_d = din("x8", [D, BC], fp8)              # fp8 x.T (ew path)
    ksel_d = din("ksel", [SUB, NS])
    w1c_d = din("w1c", [D, D], bf16)
    w1e_d = din("w1e", [D, D], fp8)             # pre-scaled by EW_WS
    w1wh_d = din("w1wh", [D, D], bf16)          # we w1 hi
    w1wl_d = din("w1wl", [D, D], bf16)          # we w1 lo residual
    b1_d = {m: din(f"b1{m}", [128, MT1]) for m in "cwe"}
    w2c_d = din("w2c", [D, EC], bf16)           # class-major column order
    w2e_d = din("w2e", [D, E], bf16)
    w2w_d = din("w2w", [D, E])                  # fp32
    b2c_d = din("b2c", [E, MT2])                # [e, class]
    b2w_d = din("b2w", [E, 1])
    b2e_d = din("b2e", [E, 1])
    hmat_d = din("hmat", [E, MT2, C], bf16)
    ident_d = din("ident", [128, 128])
    iota_d = din("iota", [128, 128])
    out_d = nc.dram_tensor("out", [C, BC], f32, kind="ExternalOutput")

    with tile.TileContext(nc) as tc, ExitStack() as ctx:
        const = ctx.enter_context(tc.tile_pool(name="const", bufs=1))
        xtp = ctx.enter_context(tc.tile_pool(name="xtp", bufs=2))
        hp = ctx.enter_context(tc.tile_pool(name="hp", bufs=2))
        epp = ctx.enter_context(tc.tile_pool(name="epp", bufs=2))
        wep = ctx.enter_context(tc.tile_pool(name="wep", bufs=2))
        sp = ctx.enter_context(tc.tile_pool(name="sp", bufs=2))
        wp = ctx.enter_context(tc.tile_pool(name="wp", bufs=2))
        psmm = ctx.enter_context(tc.tile_pool(name="psmm", bufs=4, space="PSUM"))
        pstr = ctx.enter_context(tc.tile_pool(name="pstr", bufs=1, space="PSUM"))
        psmask = ctx.enter_context(tc.tile_pool(name="psmask", bufs=1, space="PSUM"))
        pss = ctx.enter_context(tc.tile_pool(name="pss", bufs=1, space="PSUM"))
        psout = ctx.enter_context(tc.tile_pool(name="psout", bufs=1, space="PSUM"))

        # ---- resident constants / weights -------------------------------
        # Weights ride the gpsimd (SWDGE) queues so the per-tile x DMAs on
        # the sync (HWDGE) queues are not stuck behind ~10 MB of weights;
        # the first layer-1 matmul only needs w1c + the first x tile.
        # Split per k-tile so the first matmul only waits on its own slice.
        def load_w(dram, cols, dtype, tag):
            t = const.tile([128, KT, cols], dtype, tag=tag)
            ap = dram.ap().rearrange("(ko p) m -> p ko m", p=128)
            for k in range(KT):
                nc.gpsimd.dma_start(t[:, k, :], ap[:, k, :])
            return t

        def load_c(dram, shape, dtype, tag):
            t = const.tile(shape, dtype, tag=tag)
            nc.gpsimd.dma_start(t[:], dram.ap())
            return t

        # load order mirrors first-use order: we path feeds the pipe first
        w1sb = {"wh": load_w(w1wh_d, D, bf16, "w1wh")}
        w1sb["wl"] = load_w(w1wl_d, D, bf16, "w1wl")
        w2sb = {"w": load_w(w2w_d, E, f32, "w2w")}
        w1sb["c"] = load_w(w1c_d, D, bf16, "w1c")
        w1sb["e"] = load_w(w1e_d, D, fp8, "w1e")
        b1sb = {m: load_c(b1_d[m], [128, MT1], f32, f"b1{m}") for m in "cwe"}
        w2sb["c"] = load_w(w2c_d, EC, bf16, "w2c")
        w2sb["e"] = load_w(w2e_d, E, bf16, "w2e")
        b2csb = load_c(b2c_d, [E, MT2], f32, "b2c")
        b2wsb = load_c(b2w_d, [E, 1], f32, "b2w")
        b2esb = load_c(b2e_d, [E, 1], f32, "b2e")
        hmat = load_c(hmat_d, [E, MT2, C], bf16, "hmat")
        ident = load_c(ident_d, [128, 128], f32, "ident")
        iotam = load_c(iota_d, [128, 128], f32, "iota")
        kselsb = load_c(ksel_d, [SUB, NS], f32, "ksel")
        ones6 = const.tile([1, C], f32, tag="ones6")
        nc.vector.memset(ones6[:], 1.0)
        ones124 = const.tile([E, 1], bf16, tag="ones124")
        nc.vector.memset(ones124[:], 1.0)
        outacc = const.tile([C, BC], f32, tag="outacc")

        xtb_ap = xtb_d.ap().rearrange("(ko p) n -> p ko n", p=128)
        xlo_ap = xlo_d.ap().rearrange("(ko p) n -> p ko n", p=128)
        x8_ap = x8_d.ap().rearrange("(ko p) n -> p ko n", p=128)

        for T in range(NT):
            rs = slice(T * RT, (T + 1) * RT)
            xtb = xtp.tile([128, KT, RT], bf16, tag="xtb")
            nc.sync.dma_start(xtb[:], xtb_ap[:, :, rs])
            xlo = xtp.tile([128, KT, RT], bf16, tag="xlo")
            nc.sync.dma_start(xlo[:], xlo_ap[:, :, rs])
            x8 = xtp.tile([128, KT, RT], fp8, tag="x8")
            nc.sync.dma_start(x8[:], x8_ap[:, :, rs])

            expP = epp.tile([E, MT2, RT], bf16, tag="expP")
            weT = wep.tile([E, RT], f32, tag="weT")
            expew = wep.tile([E, RT], f32, tag="expew")

            # ---- we path first so the sort/mask chain overlaps the other
            # MLPs; layer 1 via bf16 hi/lo 3-matmul split (fp32 accuracy)
            htw = hp.tile([128, KT, RT], f32, tag="htw")
            for mt in range(MT1):
                cs = slice(mt * 128, (mt + 1) * 128)
                ps = psmm.tile([128, RT], f32, tag="psmm")
                for k in range(KT):
                    nc.tensor.matmul(
                        ps[:], w1sb["wh"][:, k, cs], xtb[:, k, :],
                        start=(k == 0), stop=False,
                    )
                    nc.tensor.matmul(
                        ps[:], w1sb["wh"][:, k, cs], xlo[:, k, :],
                        start=False, stop=False,
                    )
                    nc.tensor.matmul(
                        ps[:], w1sb["wl"][:, k, cs], xtb[:, k, :],
                        start=False, stop=(k == KT - 1),
                    )
                nc.scalar.activation(
                    htw[:, mt, :], ps[:], getattr(AF, act),
                    bias=b1sb["w"][:, mt : mt + 1],
                )
            ps = psmm.tile([128, RT], f32, tag="psmm")
            for k in range(KT):
                nc.tensor.matmul(
                    ps[:E], w2sb["w"][:, k, :], htw[:, k, :],
                    start=(k == 0), stop=(k == KT - 1),
                )
            nc.scalar.activation(weT[:], ps[:E], AF.Identity, bias=b2wsb[:])

            # ---- per-row top-n mask (row-major subtiles) ----------------
            maskT = psmask.tile([E, RT], f32, tag="maskT")
            for j in range(RT // SUB):
                s = (RT // SUB) * T + j
                Rr, desc = sub_r[s], sub_dir[s]
                cs = slice(j * SUB, (j + 1) * SUB)
                trp = pstr.tile([128, 128], f32, tag="trp")
                nc.tensor.transpose(trp[:, :E], weT[:, cs], ident[:E, :E])
                weRow = sp.tile([128, E], f32, tag="weRow")
                nc.scalar.copy(weRow[:], trp[:, :E])
                scratch = sp.tile([128, E], f32, tag="scratch")
                if desc:
                    nc.vector.tensor_copy(scratch[:], weRow[:])
                else:
                    nc.vector.tensor_scalar_mul(scratch[:], weRow[:], -1.0)
                srt = sp.tile([128, 128], f32, tag="srt")
                for r in range(Rr):
                    nc.vector.max(out=srt[:, 8 * r : 8 * r + 8], in_=scratch[:])
                    if r < Rr - 1:
                        nc.vector.match_replace(
                            out=scratch[:],
                            in_to_replace=srt[:, 8 * r : 8 * r + 8],
                            in_values=scratch[:],
                            imm_value=NEG_FILL,
                        )
                w8 = 8 * Rr
                ohtmp = sp.tile([128, 128], f32, tag="ohtmp")
                thr = sp.tile([128, 1], f32, tag="thr")
                nc.vector.scalar_tensor_tensor(
                    out=ohtmp[:, :w8],
                    in0=iotam[:, :w8],
                    scalar=kselsb[:, s : s + 1],
                    in1=srt[:, :w8],
                    op0=OP.is_equal,
                    op1=OP.mult,
                    accum_out=thr[:],
                )
                if not desc:
                    nc.vector.tensor_scalar_mul(thr[:], thr[:], -1.0)
                maskRow = sp.tile([128, E], f32, tag="maskRow")
                nc.vector.tensor_scalar(maskRow[:], weRow[:], thr[:], None, OP.is_ge)
                nc.tensor.transpose(maskT[:, cs], maskRow[:], ident[:])

            # ---- cls layer 1 (bf16, gelu table still resident) ----------
            htc = hp.tile([128, KT, RT], bf16, tag="htc")
            for mt in range(MT1):
                cs = slice(mt * 128, (mt + 1) * 128)
                ps = psmm.tile([128, RT], f32, tag="psmm")
                for k in range(KT):
                    nc.tensor.matmul(
                        ps[:], w1sb["c"][:, k, cs], xtb[:, k, :],
                        start=(k == 0), stop=(k == KT - 1),
                    )
                nc.scalar.activation(
                    htc[:, mt, :], ps[:], getattr(AF, act),
                    bias=b1sb["c"][:, mt : mt + 1],
                )

            # ---- ew layer 1 (fp8 DoubleRow, weights pre-scaled) ---------
            hte = hp.tile([128, KT, RT], bf16, tag="hte")
            for mt in range(MT1):
                cs = slice(mt * 128, (mt + 1) * 128)
                ps = psmm.tile([128, RT], f32, tag="psmm")
                for k2 in range(KT // 2):
                    nc.tensor.matmul(
                        ps[:],
                        w1sb["e"][:, 2 * k2 : 2 * k2 + 2, cs],
                        x8[:, 2 * k2 : 2 * k2 + 2, :],
                        start=(k2 == 0), stop=(k2 == KT // 2 - 1),
                        perf_mode=mybir.MatmulPerfMode.DoubleRow,
                    )
                nc.scalar.activation(
                    hte[:, mt, :], ps[:], getattr(AF, act),
                    bias=b1sb["e"][:, mt : mt + 1], scale=1.0 / EW_WS,
                )

            # ---- layer 2s (exp table) -----------------------------------
            for t2 in range(MT2):
                ps = psmm.tile([128, RT], f32, tag="psmm")
                for k in range(KT):
                    nc.tensor.matmul(
                        ps[:E],
                        w2sb["c"][:, k, t2 * E : (t2 + 1) * E],
                        htc[:, k, :],
                        start=(k == 0), stop=(k == KT - 1),
                    )
                nc.scalar.activation(
                    expP[:, t2, :], ps[:E], AF.Exp, bias=b2csb[:, t2 : t2 + 1]
                )
            ps = psmm.tile([128, RT], f32, tag="psmm")
            for k in range(KT):
                nc.tensor.matmul(
                    ps[:E], w2sb["e"][:, k, :], hte[:, k, :],
                    start=(k == 0), stop=(k == KT - 1),
                )
            nc.scalar.activation(expew[:], ps[:E], AF.Exp, bias=b2esb[:])

            # ---- combine (class-major expP: tile t2 = class c) ----------
            wT = wp.tile([E, RT], bf16, tag="wT")
            nc.vector.tensor_tensor(wT[:], expew[:], maskT[:], OP.mult)
            # S[e, r] = sum_c exp(logit_{e,c,r}) via a DVE add tree
            s01 = wp.tile([E, RT], f32, tag="s01")
            nc.vector.tensor_tensor(s01[:], expP[:, 0, :], expP[:, 1, :], OP.add)
            s23 = wp.tile([E, RT], f32, tag="s23")
            nc.vector.tensor_tensor(s23[:], expP[:, 2, :], expP[:, 3, :], OP.add)
            s45 = wp.tile([E, RT], f32, tag="s45")
            nc.vector.tensor_tensor(s45[:], expP[:, 4, :], expP[:, 5, :], OP.add)
            s0123 = wp.tile([E, RT], f32, tag="s0123")
            nc.vector.tensor_tensor(s0123[:], s01[:], s23[:], OP.add)
            S_sb = wp.tile([E, RT], f32, tag="S_sb")
            nc.vector.tensor_tensor(S_sb[:], s0123[:], s45[:], OP.add)
            den_ps = pss.tile([E, RT], f32, tag="S")
            nc.tensor.matmul(den_ps[:1, :], ones124[:], wT[:], start=True, stop=True)
            Sr = wp.tile([E, RT], f32, tag="Sr")
            nc.vector.reciprocal_approx_fast(Sr[:], S_sb[:])
            u = wp.tile([E, RT], bf16, tag="u")
            nc.vector.tensor_tensor(u[:], wT[:], Sr[:], OP.mult)
            out_ps = psout.tile([C, RT], f32, tag="out")
            for t2 in range(MT2):
                wexp = wp.tile([E, RT], bf16, tag="wexp")
                nc.vector.tensor_tensor(wexp[:], expP[:, t2, :], u[:], OP.mult)
                nc.tensor.matmul(
                    out_ps[:],
                    hmat[:, t2, :],
                    wexp[:],
                    start=(t2 == 0),
                    stop=(t2 == MT2 - 1),
                )
            nc.scalar.copy(outacc[:, rs], out_ps[:])

            # normalize by the expert-weight sum and ship this tile's rows
            den_sb = wp.tile([1, RT], f32, tag="den_sb")
            nc.scalar.copy(den_sb[:], den_ps[:1, :])
            recipd = wp.tile([1, RT], f32, tag="recipd")
            nc.vector.reciprocal_approx_fast(recipd[:], den_sb[:])
            rep = pss.tile([E, RT], f32, tag="S")
            nc.tensor.matmul(rep[:C, :], ones6[:], recipd[:], start=True, stop=True)
            nc.vector.tensor_tensor(outacc[:, rs], outacc[:, rs], rep[:C, :], OP.mult)
            nc.sync.dma_start(out_d.ap()[:, rs], outacc[:, rs])

    nc.compile()
    return nc


def _get_nc(sub_dir, sub_r, act="Gelu"):
    key = (tuple(sub_dir), tuple(sub_r), act)
    if key not in _BUILD_CACHE:
        _BUILD_CACHE[key] = _build_nc(sub_dir, sub_r, act)
    return _BUILD_CACHE[key]


def _host_prep(x, n_experts):
    n = np.minimum(np.asarray(n_experts).astype(np.int64), E).astype(np.int32)
    order = np.argsort(n, kind="stable")
    ns_sorted = n[order]

    sub_dir, sub_r = SUB_DIR, SUB_R
    ok = True
    for s in range(NS):
        lo = int(ns_sorted[(B // NS) * s])
        hi = int(ns_sorted[(B // NS) * (s + 1) - 1])
        if sub_dir[s]:
            ok &= hi <= 8 * sub_r[s]
        else:
            ok &= lo >= E + 1 - 8 * sub_r[s]
    if not ok:
        sub_dir, sub_r = FALLBACK_DIR, FALLBACK_R

    rows_by_core = [order[c::NCORES] for c in range(NCORES)]
    import ml_dtypes

    bf16 = ml_dtypes.bfloat16
    fp8 = ml_dtypes.float8_e4m3fn
    xts, ksels = [], []
    for c in range(NCORES):
        rows = rows_by_core[c]
        xt = np.ascontiguousarray(x[rows].T.astype(np.float32))
        xhi = xt.astype(bf16)
        xlo = (xt - xhi.astype(np.float32)).astype(bf16)
        xts.append((xhi, xlo, xt.astype(fp8)))
        nv = n[rows].astype(np.float32)
        ks = np.empty(BC, np.float32)
        for s in range(NS):
            seg = slice(SUB * s, SUB * (s + 1))
            ks[seg] = (nv[seg] - 1.0) if sub_dir[s] else (E - nv[seg])
        ksels.append(np.ascontiguousarray(ks.reshape(NS, SUB).T))
    return rows_by_core, xts, ksels, sub_dir, sub_r


def _host_consts():
    # class-major cls tiling: output tile t holds columns f' = t*E + e,
    # which map to original cls column e*C + t (expert e, class t)
    hmat = np.zeros((E, MT2, C), np.float32)
    for t in range(MT2):
        hmat[:, t, t] = 1.0
    ident = np.eye(128, dtype=np.float32)
    iota = np.broadcast_to(np.arange(128, dtype=np.float32), (128, 128)).copy()
    return hmat, ident, iota


def _host_inputs(inputs):
    """All DRAM input arrays except the per-core xt/ksel."""
    import ml_dtypes

    bf16 = ml_dtypes.bfloat16
    hmat, ident, iota = _host_consts()
    f32 = np.float32
    w1w = np.asarray(inputs["we_w1"], f32)
    w1wh = w1w.astype(bf16)
    w1wl = (w1w - w1wh.astype(f32)).astype(bf16)
    # permute cls layer-2 columns to class-major: new col t*E+e <- e*C+t
    cidx = (np.arange(C)[:, None] + (np.arange(E) * C)[None, :]).ravel()
    w2c = np.asarray(inputs["cls_w2"], f32)[:, cidx]
    b2c = np.asarray(inputs["cls_b2"], f32)[cidx]
    return {
        "w1c": np.asarray(inputs["cls_w1"], f32).astype(bf16),
        "w1wh": w1wh,
        "w1wl": w1wl,
        "w1e": (np.asarray(inputs["ew_w1"], f32) * 16.0).astype(
            ml_dtypes.float8_e4m3fn
        ),
        "b1c": np.ascontiguousarray(np.asarray(inputs["cls_b1"], f32).reshape(MT1, 128).T),
        "b1w": np.ascontiguousarray(np.asarray(inputs["we_b1"], f32).reshape(MT1, 128).T),
        "b1e": np.ascontiguousarray(np.asarray(inputs["ew_b1"], f32).reshape(MT1, 128).T),
        "w2c": w2c.astype(bf16),
        "w2w": np.asarray(inputs["we_w2"], f32),
        "w2e": np.asarray(inputs["ew_w2"], f32).astype(bf16),
        "b2c": np.ascontiguousarray(b2c.reshape(MT2, E).T),
        "b2w": np.asarray(inputs["we_b2"], f32).reshape(E, 1),
        "b2e": np.asarray(inputs["ew_b2"], f32).reshape(E, 1),
        "hmat": hmat.astype(bf16),
        "ident": ident,
        "iota": iota,
    }


def _per_core_inputs(xts, ksels, c):
    return {
        "xtb": xts[c][0],
        "xlo": xts[c][1],
        "x8": xts[c][2],
        "ksel": ksels[c],
    }


def kernel(**inputs):
    x = np.asarray(inputs["x"], np.float32)
    rows_by_core, xts, ksels, sub_dir, sub_r = _host_prep(x, inputs["n_experts"])
    shared = _host_inputs(inputs)
    in_maps = [
        {**shared, **_per_core_inputs(xts, ksels, c)} for c in range(NCORES)
    ]

    nc = _get_nc(sub_dir, sub_r)

    from concourse.bass_utils import run_bass_kernel_spmd

    res = run_bass_kernel_spmd(nc, in_maps, core_ids=list(range(NCORES)))

    full = np.empty((B, C), np.float32)
    for c in range(NCORES):
        full[rows_by_core[c]] = res.results[c]["out"].T
    return full


if __name__ == "__main__":
    print("smoke build only")
    _get_nc(SUB_DIR, SUB_R)
    print("built ok")



# revision 45
# speedup vs baseline: 1.4070x; 1.0892x over previous
"""Trainium2 Bass kernel for nn_ExpertsLinearEnsemble.

Reference computation (B=16384, D=768, E=124, C=6):
  expert_logits  = Mlp_cls(x).reshape(B, E, C)     # D -> D -> gelu -> E*C
  ew_logits      = Mlp_ew(x)                       # D -> D -> gelu -> E
  which_expert   = Mlp_we(x)                       # D -> D -> gelu -> E
  n = clamp(n_experts, E); thr = n-th largest of which_expert per row
  mask out experts with which_expert < thr; softmax ew_logits over kept
  experts; softmax expert_logits over classes; combined = sum_e w_e *
  proba_e / sum_e w_e.

Strategy (pure data parallel, 2048 rows/core):
  - Host transposes x so the contraction dim D sits on SBUF partitions;
    the whole device pipeline runs feature-major ([feature, row] tiles),
    so matmul outputs feed the next layer with no on-chip transposes of
    activations.
  - Precision split: the which_expert path must reproduce the fp32
    ordering of the reference (the top-n mask is a hard threshold), so
    it runs in plain fp32 matmuls.  The cls/ew paths and the small
    combine matmuls only need ~1e-3 relative accuracy and run in bf16
    (fp32 PSUM accumulation).
  - Top-n threshold per row: rows are sorted by n on host and dealt
    round-robin to cores, so each 128-row subtile has a narrow n-range
    and a fixed number of max8/match_replace rounds (descending sort for
    small n, ascending for large n) suffices.  The threshold is the
    (n-1)-th sorted value, extracted with a one-hot dot against a host
    supplied selector index; mask = which_expert >= thr matches the
    reference's `which_expert < thr` masking exactly, ties included.
  - Class softmax sums / per-expert broadcasts / final combine run as
    tiny PE matmuls against constant group matrices; the expert-weight
    sum (softmax denominator, which algebraically cancels against the
    final division) accumulates in a separate row and one divide at the
    end normalizes everything.
"""

import os
import sys

for _p in ("/opt/trn_rl_repo", "/root/.axon_site/_ro/trn_rl_repo"):
    if os.path.isdir(_p) and _p not in sys.path:
        sys.path.insert(0, _p)

import numpy as np

B, D, E, C = 16384, 768, 124, 6
EC = E * C            # 744
NCORES = 8
BC = B // NCORES      # 2048 rows per core
RT = 512              # rows per macro tile (PSUM bank = 512 fp32)
NT = BC // RT         # 4 macro tiles per core
SUB = 128             # rows per sort subtile
NS = BC // SUB        # 16 subtiles per core
KT = D // 128         # 6 contraction tiles
MT1 = D // 128        # 6 output tiles for layer 1
MT2 = EC // E         # 6 output tiles of 124 for the cls head

# Fixed per-subtile sort schedule.  Subtile s holds rows whose global
# sorted-n positions are [1024 s, 1024 (s+1)); with n ~ U[1,124] the
# boundary quantiles sit many sigma inside these capabilities.
# s in [0, 8): descending sort, handles n <= 8 R; s in [8, 16): ascending
# (sort of -we), handles n >= 125 - 8 R.
R_DESC = [2, 3, 4, 5, 6, 7, 8, 9]
R_ASC = [9, 8, 7, 6, 5, 4, 3, 2]
SUB_DIR = [True] * 8 + [False] * 8         # True = descending
SUB_R = R_DESC + R_ASC
FALLBACK_R = [16] * NS                     # safe for any n distribution
FALLBACK_DIR = [True] * NS

NEG_FILL = -1.0e30

_BUILD_CACHE = {}


def _build_nc(sub_dir, sub_r, act="Gelu"):
    """Build the (SPMD, per-core) Bass program.  Data independent."""
    from contextlib import ExitStack

    import concourse.mybir as mybir
    import concourse.tile as tile
    from concourse import bacc

    dt = mybir.dt
    AF = mybir.ActivationFunctionType
    OP = mybir.AluOpType
    f32 = dt.float32
    f32r = dt.float32r
    bf16 = dt.bfloat16

    nc = bacc.Bacc(
        "TRN2",
        target_bir_lowering=False,
        debug=False,
        enable_asserts=False,
        num_devices=NCORES,
    )

    def din(name, shape, dtype=f32):
        return nc.dram_tensor(name, list(shape), dtype, kind="ExternalInput")

    fp8 = dt.float8e4
    f16 = dt.float16
    EW_WS = 16.0                                # ew weight pre-scale (fp8 range)
    WE_SC = 2.0 ** -13                          # we hi/lo common PSUM scale

    x16_d = din("x16", [D, BC], f16)            # fp16 x.T (cls + we main)
    x8_d = din("x8", [D, BC], fp8)              # fp8 x.T (ew path + we crossA)
    x8r_d = din("x8r", [D, BC], fp8)            # fp8 (x - fp16(x))*2^11 (we crossB)
    ksel_d = din("ksel", [SUB, NS])
    w1c_d = din("w1c", [D, D], f16)
    w1e_d = din("w1e", [D, D], fp8)             # pre-scaled by EW_WS
    w1w16_d = din("w1w16", [D, D], f16)         # fp16(w1w) * 2^13
    w1wr8_d = din("w1wr8", [D, D], fp8)         # (w1w - fp16(w1w)) * 2^13
    w1wh8_d = din("w1wh8", [D, D], fp8)         # w1w * 2^2
    b1_d = {m: din(f"b1{m}", [128, MT1]) for m in "cwe"}
    w2c_d = din("w2c", [D, EC], f16)            # class-major column order
    w2e_d = din("w2e", [D, E], f16)
    w2w_d = din("w2w", [D, E])                  # fp32
    b2c_d = din("b2c", [E, MT2])                # [e, class]
    b2w_d = din("b2w", [E, 1])
    b2e_d = din("b2e", [E, 1])
    hmat_d = din("hmat", [E, MT2, C], bf16)
    ident_d = din("ident", [128, 128])
    iota_d = din("iota", [128, 128])
    out_d = nc.dram_tensor("out", [C, BC], f32, kind="ExternalOutput")

    with tile.TileContext(nc) as tc, ExitStack() as ctx:
        const = ctx.enter_context(tc.tile_pool(name="const", bufs=1))
        xtp = ctx.enter_context(tc.tile_pool(name="xtp", bufs=2))
        hp = ctx.enter_context(tc.tile_pool(name="hp", bufs=2))
        epp = ctx.enter_context(tc.tile_pool(name="epp", bufs=2))
        wep = ctx.enter_context(tc.tile_pool(name="wep", bufs=2))
        sp = ctx.enter_context(tc.tile_pool(name="sp", bufs=2))
        wp = ctx.enter_context(tc.tile_pool(name="wp", bufs=2))
        psmm = ctx.enter_context(tc.tile_pool(name="psmm", bufs=4, space="PSUM"))
        pstr = ctx.enter_context(tc.tile_pool(name="pstr", bufs=1, space="PSUM"))
        psmask = ctx.enter_context(tc.tile_pool(name="psmask", bufs=1, space="PSUM"))
        pss = ctx.enter_context(tc.tile_pool(name="pss", bufs=1, space="PSUM"))
        psout = ctx.enter_context(tc.tile_pool(name="psout", bufs=1, space="PSUM"))

        # ---- resident constants / weights -------------------------------
        # Weights ride the gpsimd (SWDGE) queues so the per-tile x DMAs on
        # the sync (HWDGE) queues are not stuck behind ~10 MB of weights;
        # the first layer-1 matmul only needs w1c + the first x tile.
        # Split per k-tile so the first matmul only waits on its own slice.
        def load_w(dram, cols, dtype, tag):
            t = const.tile([128, KT, cols], dtype, tag=tag)
            ap = dram.ap().rearrange("(ko p) m -> p ko m", p=128)
            for k in range(KT):
                nc.gpsimd.dma_start(t[:, k, :], ap[:, k, :])
            return t

        def load_c(dram, shape, dtype, tag):
            t = const.tile(shape, dtype, tag=tag)
            nc.gpsimd.dma_start(t[:], dram.ap())
            return t

        # load order mirrors first-use order: we path feeds the pipe first
        w1sb = {"w16": load_w(w1w16_d, D, f16, "w1w16")}
        w1sb["wr8"] = load_w(w1wr8_d, D, fp8, "w1wr8")
        w1sb["wh8"] = load_w(w1wh8_d, D, fp8, "w1wh8")
        w2sb = {"w": load_w(w2w_d, E, f32, "w2w")}
        w1sb["c"] = load_w(w1c_d, D, f16, "w1c")
        w1sb["e"] = load_w(w1e_d, D, fp8, "w1e")
        b1sb = {m: load_c(b1_d[m], [128, MT1], f32, f"b1{m}") for m in "cwe"}
        w2sb["c"] = load_w(w2c_d, EC, f16, "w2c")
        w2sb["e"] = load_w(w2e_d, E, f16, "w2e")
        b2csb = load_c(b2c_d, [E, MT2], f32, "b2c")
        b2wsb = load_c(b2w_d, [E, 1], f32, "b2w")
        b2esb = load_c(b2e_d, [E, 1], f32, "b2e")
        hmat = load_c(hmat_d, [E, MT2, C], bf16, "hmat")
        ident = load_c(ident_d, [128, 128], f32, "ident")
        iotam = load_c(iota_d, [128, 128], f32, "iota")
        kselsb = load_c(ksel_d, [SUB, NS], f32, "ksel")
        ones6 = const.tile([1, C], f32, tag="ones6")
        nc.vector.memset(ones6[:], 1.0)
        ones124 = const.tile([E, 1], bf16, tag="ones124")
        nc.vector.memset(ones124[:], 1.0)
        outacc = const.tile([C, BC], f32, tag="outacc")

        x16_ap = x16_d.ap().rearrange("(ko p) n -> p ko n", p=128)
        x8_ap = x8_d.ap().rearrange("(ko p) n -> p ko n", p=128)
        x8r_ap = x8r_d.ap().rearrange("(ko p) n -> p ko n", p=128)

        for T in range(NT):
            rs = slice(T * RT, (T + 1) * RT)
            x16 = xtp.tile([128, KT, RT], f16, tag="x16")
            nc.sync.dma_start(x16[:], x16_ap[:, :, rs])
            x8 = xtp.tile([128, KT, RT], fp8, tag="x8")
            nc.sync.dma_start(x8[:], x8_ap[:, :, rs])
            x8r = xtp.tile([128, KT, RT], fp8, tag="x8r")
            nc.sync.dma_start(x8r[:], x8r_ap[:, :, rs])

            expP = epp.tile([E, MT2, RT], bf16, tag="expP")
            weT = wep.tile([E, RT], f32, tag="weT")
            expew = wep.tile([E, RT], f32, tag="expew")

            # ---- we path first so the sort/mask chain overlaps the other
            # MLPs.  Layer 1 at full fp32 accuracy via a single 2^13-scaled
            # PSUM group: fp16 main (x16 @ fp16(w)*2^13) plus two fp8
            # DoubleRow residual corrections; activation scale undoes 2^13.
            htw = hp.tile([128, KT, RT], f32, tag="htw")
            for mt in range(MT1):
                cs = slice(mt * 128, (mt + 1) * 128)
                ps = psmm.tile([128, RT], f32, tag="psmm")
                for k in range(KT):
                    nc.tensor.matmul(
                        ps[:], w1sb["w16"][:, k, cs], x16[:, k, :],
                        start=(k == 0), stop=False,
                    )
                for k2 in range(KT // 2):
                    nc.tensor.matmul(
                        ps[:],
                        w1sb["wr8"][:, 2 * k2 : 2 * k2 + 2, cs],
                        x8[:, 2 * k2 : 2 * k2 + 2, :],
                        start=False, stop=False,
                        perf_mode=mybir.MatmulPerfMode.DoubleRow,
                    )
                for k2 in range(KT // 2):
                    nc.tensor.matmul(
                        ps[:],
                        w1sb["wh8"][:, 2 * k2 : 2 * k2 + 2, cs],
                        x8r[:, 2 * k2 : 2 * k2 + 2, :],
                        start=False, stop=(k2 == KT // 2 - 1),
                        perf_mode=mybir.MatmulPerfMode.DoubleRow,
                    )
                nc.scalar.activation(
                    htw[:, mt, :], ps[:], getattr(AF, act),
                    bias=b1sb["w"][:, mt : mt + 1], scale=WE_SC,
                )
            ps = psmm.tile([128, RT], f32, tag="psmm")
            for k in range(KT):
                nc.tensor.matmul(
                    ps[:E], w2sb["w"][:, k, :], htw[:, k, :],
                    start=(k == 0), stop=(k == KT - 1),
                )
            nc.scalar.activation(weT[:], ps[:E], AF.Identity, bias=b2wsb[:])

            # ---- per-row top-n mask (row-major subtiles) ----------------
            maskT = psmask.tile([E, RT], f32, tag="maskT")
            for j in range(RT // SUB):
                s = (RT // SUB) * T + j
                Rr, desc = sub_r[s], sub_dir[s]
                cs = slice(j * SUB, (j + 1) * SUB)
                trp = pstr.tile([128, 128], f32, tag="trp")
                nc.tensor.transpose(trp[:, :E], weT[:, cs], ident[:E, :E])
                weRow = sp.tile([128, E], f32, tag="weRow")
                nc.scalar.copy(weRow[:], trp[:, :E])
                scratch = sp.tile([128, E], f32, tag="scratch")
                if desc:
                    nc.vector.tensor_copy(scratch[:], weRow[:])
                else:
                    nc.vector.tensor_scalar_mul(scratch[:], weRow[:], -1.0)
                srt = sp.tile([128, 128], f32, tag="srt")
                for r in range(Rr):
                    nc.vector.max(out=srt[:, 8 * r : 8 * r + 8], in_=scratch[:])
                    if r < Rr - 1:
                        nc.vector.match_replace(
                            out=scratch[:],
                            in_to_replace=srt[:, 8 * r : 8 * r + 8],
                            in_values=scratch[:],
                            imm_value=NEG_FILL,
                        )
                w8 = 8 * Rr
                ohtmp = sp.tile([128, 128], f32, tag="ohtmp")
                thr = sp.tile([128, 1], f32, tag="thr")
                nc.vector.scalar_tensor_tensor(
                    out=ohtmp[:, :w8],
                    in0=iotam[:, :w8],
                    scalar=kselsb[:, s : s + 1],
                    in1=srt[:, :w8],
                    op0=OP.is_equal,
                    op1=OP.mult,
                    accum_out=thr[:],
                )
                if not desc:
                    nc.vector.tensor_scalar_mul(thr[:], thr[:], -1.0)
                maskRow = sp.tile([128, E], f32, tag="maskRow")
                nc.vector.tensor_scalar(maskRow[:], weRow[:], thr[:], None, OP.is_ge)
                nc.tensor.transpose(maskT[:, cs], maskRow[:], ident[:])

            # ---- cls layer 1 (fp16, gelu table still resident) ----------
            htc = hp.tile([128, KT, RT], f16, tag="htc")
            for mt in range(MT1):
                cs = slice(mt * 128, (mt + 1) * 128)
                ps = psmm.tile([128, RT], f32, tag="psmm")
                for k in range(KT):
                    nc.tensor.matmul(
                        ps[:], w1sb["c"][:, k, cs], x16[:, k, :],
                        start=(k == 0), stop=(k == KT - 1),
                    )
                nc.scalar.activation(
                    htc[:, mt, :], ps[:], getattr(AF, act),
                    bias=b1sb["c"][:, mt : mt + 1],
                )

            # ---- ew layer 1 (fp8 DoubleRow, weights pre-scaled) ---------
            hte = hp.tile([128, KT, RT], f16, tag="hte")
            for mt in range(MT1):
                cs = slice(mt * 128, (mt + 1) * 128)
                ps = psmm.tile([128, RT], f32, tag="psmm")
                for k2 in range(KT // 2):
                    nc.tensor.matmul(
                        ps[:],
                        w1sb["e"][:, 2 * k2 : 2 * k2 + 2, cs],
                        x8[:, 2 * k2 : 2 * k2 + 2, :],
                        start=(k2 == 0), stop=(k2 == KT // 2 - 1),
                        perf_mode=mybir.MatmulPerfMode.DoubleRow,
                    )
                nc.scalar.activation(
                    hte[:, mt, :], ps[:], getattr(AF, act),
                    bias=b1sb["e"][:, mt : mt + 1], scale=1.0 / EW_WS,
                )

            # ---- layer 2s (exp table) -----------------------------------
            for t2 in range(MT2):
                ps = psmm.tile([128, RT], f32, tag="psmm")
                for k in range(KT):
                    nc.tensor.matmul(
                        ps[:E],
                        w2sb["c"][:, k, t2 * E : (t2 + 1) * E],
                        htc[:, k, :],
                        start=(k == 0), stop=(k == KT - 1),
                    )
                nc.scalar.activation(
                    expP[:, t2, :], ps[:E], AF.Exp, bias=b2csb[:, t2 : t2 + 1]
                )
            ps = psmm.tile([128, RT], f32, tag="psmm")
            for k in range(KT):
                nc.tensor.matmul(
                    ps[:E], w2sb["e"][:, k, :], hte[:, k, :],
                    start=(k == 0), stop=(k == KT - 1),
                )
            nc.scalar.activation(expew[:], ps[:E], AF.Exp, bias=b2esb[:])

            # ---- combine (class-major expP: tile t2 = class c) ----------
            wT = wp.tile([E, RT], bf16, tag="wT")
            nc.vector.tensor_tensor(wT[:], expew[:], maskT[:], OP.mult)
            # S[e, r] = sum_c exp(logit_{e,c,r}) via a DVE add tree
            s01 = wp.tile([E, RT], f32, tag="s01")
            nc.vector.tensor_tensor(s01[:], expP[:, 0, :], expP[:, 1, :], OP.add)
            s23 = wp.tile([E, RT], f32, tag="s23")
            nc.vector.tensor_tensor(s23[:], expP[:, 2, :], expP[:, 3, :], OP.add)
            s45 = wp.tile([E, RT], f32, tag="s45")
            nc.vector.tensor_tensor(s45[:], expP[:, 4, :], expP[:, 5, :], OP.add)
            s0123 = wp.tile([E, RT], f32, tag="s0123")
            nc.vector.tensor_tensor(s0123[:], s01[:], s23[:], OP.add)
            S_sb = wp.tile([E, RT], f32, tag="S_sb")
            nc.vector.tensor_tensor(S_sb[:], s0123[:], s45[:], OP.add)
            den_ps = pss.tile([E, RT], f32, tag="S")
            nc.tensor.matmul(den_ps[:1, :], ones124[:], wT[:], start=True, stop=True)
            Sr = wp.tile([E, RT], f32, tag="Sr")
            nc.vector.reciprocal_approx_fast(Sr[:], S_sb[:])
            u = wp.tile([E, RT], bf16, tag="u")
            nc.vector.tensor_tensor(u[:], wT[:], Sr[:], OP.mult)
            out_ps = psout.tile([C, RT], f32, tag="out")
            for t2 in range(MT2):
                wexp = wp.tile([E, RT], bf16, tag="wexp")
                nc.vector.tensor_tensor(wexp[:], expP[:, t2, :], u[:], OP.mult)
                nc.tensor.matmul(
                    out_ps[:],
                    hmat[:, t2, :],
                    wexp[:],
                    start=(t2 == 0),
                    stop=(t2 == MT2 - 1),
                )
            nc.scalar.copy(outacc[:, rs], out_ps[:])

            # normalize by the expert-weight sum and ship this tile's rows
            den_sb = wp.tile([1, RT], f32, tag="den_sb")
            nc.scalar.copy(den_sb[:], den_ps[:1, :])
            recipd = wp.tile([1, RT], f32, tag="recipd")
            nc.vector.reciprocal_approx_fast(recipd[:], den_sb[:])
            rep = pss.tile([E, RT], f32, tag="S")
            nc.tensor.matmul(rep[:C, :], ones6[:], recipd[:], start=True, stop=True)
            nc.vector.tensor_tensor(outacc[:, rs], outacc[:, rs], rep[:C, :], OP.mult)
            nc.sync.dma_start(out_d.ap()[:, rs], outacc[:, rs])

    nc.compile()
    return nc


def _get_nc(sub_dir, sub_r, act="Gelu"):
    key = (tuple(sub_dir), tuple(sub_r), act)
    if key not in _BUILD_CACHE:
        _BUILD_CACHE[key] = _build_nc(sub_dir, sub_r, act)
    return _BUILD_CACHE[key]


def _host_prep(x, n_experts):
    n = np.minimum(np.asarray(n_experts).astype(np.int64), E).astype(np.int32)
    order = np.argsort(n, kind="stable")
    ns_sorted = n[order]

    sub_dir, sub_r = SUB_DIR, SUB_R
    ok = True
    for s in range(NS):
        lo = int(ns_sorted[(B // NS) * s])
        hi = int(ns_sorted[(B // NS) * (s + 1) - 1])
        if sub_dir[s]:
            ok &= hi <= 8 * sub_r[s]
        else:
            ok &= lo >= E + 1 - 8 * sub_r[s]
    if not ok:
        sub_dir, sub_r = FALLBACK_DIR, FALLBACK_R

    rows_by_core = [order[c::NCORES] for c in range(NCORES)]
    import ml_dtypes

    fp8 = ml_dtypes.float8_e4m3fn
    xts, ksels = [], []
    for c in range(NCORES):
        rows = rows_by_core[c]
        xt = np.ascontiguousarray(x[rows].T.astype(np.float32))
        x16 = xt.astype(np.float16)
        x8r = ((xt - x16.astype(np.float32)) * 2.0**11).astype(fp8)
        xts.append((x16, xt.astype(fp8), x8r))
        nv = n[rows].astype(np.float32)
        ks = np.empty(BC, np.float32)
        for s in range(NS):
            seg = slice(SUB * s, SUB * (s + 1))
            ks[seg] = (nv[seg] - 1.0) if sub_dir[s] else (E - nv[seg])
        ksels.append(np.ascontiguousarray(ks.reshape(NS, SUB).T))
    return rows_by_core, xts, ksels, sub_dir, sub_r


def _host_consts():
    # class-major cls tiling: output tile t holds columns f' = t*E + e,
    # which map to original cls column e*C + t (expert e, class t)
    hmat = np.zeros((E, MT2, C), np.float32)
    for t in range(MT2):
        hmat[:, t, t] = 1.0
    ident = np.eye(128, dtype=np.float32)
    iota = np.broadcast_to(np.arange(128, dtype=np.float32), (128, 128)).copy()
    return hmat, ident, iota


def _host_inputs(inputs):
    """All DRAM input arrays except the per-core xt/ksel."""
    import ml_dtypes

    fp8 = ml_dtypes.float8_e4m3fn
    f16 = np.float16
    hmat, ident, iota = _host_consts()
    f32 = np.float32
    w1w = np.asarray(inputs["we_w1"], f32)
    w1w16 = w1w.astype(f16)
    # permute cls layer-2 columns to class-major: new col t*E+e <- e*C+t
    cidx = (np.arange(C)[:, None] + (np.arange(E) * C)[None, :]).ravel()
    w2c = np.asarray(inputs["cls_w2"], f32)[:, cidx]
    b2c = np.asarray(inputs["cls_b2"], f32)[cidx]
    return {
        "w1c": np.asarray(inputs["cls_w1"], f32).astype(f16),
        "w1w16": (w1w16.astype(f32) * 2.0**13).astype(f16),
        "w1wr8": ((w1w - w1w16.astype(f32)) * 2.0**13).astype(fp8),
        "w1wh8": (w1w * 2.0**2).astype(fp8),
        "w1e": (np.asarray(inputs["ew_w1"], f32) * 16.0).astype(fp8),
        "b1c": np.ascontiguousarray(np.asarray(inputs["cls_b1"], f32).reshape(MT1, 128).T),
        "b1w": np.ascontiguousarray(np.asarray(inputs["we_b1"], f32).reshape(MT1, 128).T),
        "b1e": np.ascontiguousarray(np.asarray(inputs["ew_b1"], f32).reshape(MT1, 128).T),
        "w2c": w2c.astype(f16),
        "w2w": np.asarray(inputs["we_w2"], f32),
        "w2e": np.asarray(inputs["ew_w2"], f32).astype(f16),
        "b2c": np.ascontiguousarray(b2c.reshape(MT2, E).T),
        "b2w": np.asarray(inputs["we_b2"], f32).reshape(E, 1),
        "b2e": np.asarray(inputs["ew_b2"], f32).reshape(E, 1),
        "hmat": hmat.astype(ml_dtypes.bfloat16),
        "ident": ident,
        "iota": iota,
    }


def _per_core_inputs(xts, ksels, c):
    return {
        "x16": xts[c][0],
        "x8": xts[c][1],
        "x8r": xts[c][2],
        "ksel": ksels[c],
    }


def kernel(**inputs):
    x = np.asarray(inputs["x"], np.float32)
    rows_by_core, xts, ksels, sub_dir, sub_r = _host_prep(x, inputs["n_experts"])
    shared = _host_inputs(inputs)
    in_maps = [
        {**shared, **_per_core_inputs(xts, ksels, c)} for c in range(NCORES)
    ]

    nc = _get_nc(sub_dir, sub_r)

    from concourse.bass_utils import run_bass_kernel_spmd

    res = run_bass_kernel_spmd(nc, in_maps, core_ids=list(range(NCORES)))

    full = np.empty((B, C), np.float32)
    for c in range(NCORES):
        full[rows_by_core[c]] = res.results[c]["out"].T
    return full


if __name__ == "__main__":
    print("smoke build only")
    _get_nc(SUB_DIR, SUB_R)
    print("built ok")

